# revision 5
# baseline (speedup 1.0000x reference)
"""Sliding-window multi-head attention (B=2, S=2048, D=E=768, H=12, window/2=128)
as a Bass/Tile kernel on 8 Trainium2 NeuronCores.

Sharding: data-parallel over batch (2) x tensor-parallel over heads (4 groups
of 3 heads).  Core c handles batch c//4, heads [3*(c%4) .. 3*(c%4)+2].
Each core computes its heads' QKV projection, banded attention, and a partial
output projection (contraction over its 192 features of E); the host sums the
4 partials per batch and adds bo.

Device dataflow (per core), everything fp32:
  phase 1: qk^T feature-major via W-stationary matmuls; V token-major via
           xT-stationary matmuls (ones column appended -> V_aug).
  phase 2: per (head, key-block kb): scores^T[k,q] = K_kb @ Q^T(window)
           (1/sqrt(hd) folded into Wq on host); exp on ScalarE with the
           padding mask as per-partition bias; band mask via gpsimd
           affine_select (only the two off-diagonal 128-blocks need it);
           AV with expS^T stationary -> out[q, 65] where col 64 = softmax
           denominator; normalize via reciprocal + tensor_scalar_mul;
           PE-transpose back to feature-major vals^T.
  phase 3: partial o^T[e,t] = Wo_slice^T-stationary matmuls over 3 heads.
"""
import sys

if "/opt/trn_rl_repo" not in sys.path:
    sys.path.insert(0, "/opt/trn_rl_repo")

import numpy as np

B = 2
S = 2048
D = 768
E = 768
H = 12
HD = 64
HALF_WIN = 128  # WINDOW_SIZE // 2
N_CORES = 8
HPC = 3  # heads per core
NEG = -1e30

N_TBLK = S // 512      # 4
N_DBLK = D // 128      # 6
N_KB = S // 128        # 16
N_EBLK = E // 128      # 6

_compiled = None


def _build():
    import concourse.bass as bass
    import concourse.bacc as bacc
    import concourse.mybir as mybir
    import concourse.tile as tile
    from concourse.masks import make_identity
    from contextlib import ExitStack

    F32 = mybir.dt.float32
    AF = mybir.ActivationFunctionType

    nc = bacc.Bacc(None, target_bir_lowering=False)

    xT = nc.dram_tensor("xT", [D, S], F32, kind="ExternalInput")
    # wqk free-dim layout: [q0|q1](128) [k0|k1](128) [q2|k2](128)
    wqk = nc.dram_tensor("wqk", [128, N_DBLK, 384], F32, kind="ExternalInput")
    wv = nc.dram_tensor("wv", [128, N_DBLK, 192], F32, kind="ExternalInput")
    wo = nc.dram_tensor("wo", [64, HPC, E], F32, kind="ExternalInput")
    pmask = nc.dram_tensor("pmask", [128, N_KB], F32, kind="ExternalInput")
    oT = nc.dram_tensor("oT", [E, S], F32, kind="ExternalOutput")

    with tile.TileContext(nc) as tc, ExitStack() as ctx:
        singles = ctx.enter_context(tc.tile_pool(name="singles", bufs=1))
        xpool = ctx.enter_context(tc.tile_pool(name="xpool", bufs=2))
        epool = ctx.enter_context(tc.tile_pool(name="epool", bufs=3))
        vtpool = ctx.enter_context(tc.tile_pool(name="vtpool", bufs=3))
        rpool = ctx.enter_context(tc.tile_pool(name="rpool", bufs=3))
        ospool = ctx.enter_context(tc.tile_pool(name="ospool", bufs=3))
        mmps = ctx.enter_context(tc.tile_pool(name="mmps", bufs=4, space="PSUM"))
        ops = ctx.enter_context(tc.tile_pool(name="ops", bufs=3, space="PSUM"))
        tps = ctx.enter_context(tc.tile_pool(name="tps", bufs=1, space="PSUM"))

        # --- resident tiles ---
        wqk_sb = singles.tile([128, N_DBLK, 384], F32)
        wv_sb = singles.tile([128, N_DBLK, 192], F32)
        wo_sb = singles.tile([64, HPC, E], F32)
        pm_sb = singles.tile([128, N_KB], F32)
        ident = singles.tile([128, 128], F32)
        qT01 = singles.tile([128, S], F32)   # rows 0:64 q0, 64:128 q1
        kT01 = singles.tile([128, S], F32)   # rows 0:64 k0, 64:128 k1
        qk2 = singles.tile([128, S], F32)    # rows 0:64 q2, 64:128 k2
        q2s = singles.tile([128, S], F32)    # rows 64:128 = q2 (DMA-shifted)
        v_aug = singles.tile([128, N_KB, HPC * 65], F32)  # per kb: [v_h|1]*3
        valsT = [singles.tile([64, S], F32, name=f"valsT{h}", tag=f"valsT{h}")
                 for h in range(HPC)]

        nc.sync.dma_start(wqk_sb, wqk[:, :, :])
        nc.sync.dma_start(wv_sb, wv[:, :, :])
        nc.sync.dma_start(wo_sb, wo[:, :, :])
        nc.sync.dma_start(pm_sb, pmask[:, :])
        make_identity(nc, ident)
        ones_cols = v_aug.rearrange("p t (h c) -> p t h c", c=65)[:, :, :, 64:65]
        nc.vector.memset(ones_cols, 1.0)

        # ---------------- phase 1: QKV ----------------
        for tb in range(N_TBLK):
            ts = slice(tb * 512, (tb + 1) * 512)
            xt = xpool.tile([128, N_DBLK, 512], F32, tag="xt")
            for db in range(N_DBLK):
                nc.sync.dma_start(
                    xt[:, db, :], xT[db * 128:(db + 1) * 128, ts])
            # q/k feature-major: 3 M-tiles
            for mt, dest in ((0, qT01), (1, kT01), (2, qk2)):
                ps = mmps.tile([128, 512], F32, tag="mm")
                for db in range(N_DBLK):
                    nc.tensor.matmul(
                        ps, lhsT=wqk_sb[:, db, mt * 128:(mt + 1) * 128],
                        rhs=xt[:, db, :], start=(db == 0), stop=(db == N_DBLK - 1))
                nc.scalar.copy(dest[:, ts], ps)
            # shift q2 rows 0:64 -> q2s rows 64:128 (cross-partition via DMA)
            nc.sync.dma_start(q2s[64:128, ts], qk2[0:64, ts])
            # v token-major: 4 chunks of 128 tokens
            for i in range(4):
                tk = tb * 4 + i
                psv = mmps.tile([128, 192], F32, tag="mm")
                for db in range(N_DBLK):
                    nc.tensor.matmul(
                        psv, lhsT=xt[:, db, i * 128:(i + 1) * 128],
                        rhs=wv_sb[:, db, :], start=(db == 0), stop=(db == N_DBLK - 1))
                dst = v_aug.rearrange("p t (h c) -> p t h c", c=65)[:, tk, :, 0:64]
                src = psv.rearrange("p (h c) -> p h c", c=64)
                nc.vector.tensor_copy(dst, src)

        # ---------------- phase 2: banded attention ----------------
        def score_ops(h):
            if h == 0:
                return kT01[0:64, :], qT01[0:64, :]
            if h == 1:
                return kT01[64:128, :], qT01[64:128, :]
            return qk2[64:128, :], q2s[64:128, :]

        for h in range(HPC):
            kt_full, qt_full, = score_ops(h)
            ps_o = {}
            for kb in range(N_KB):
                w0 = max(0, kb * 128 - 128)
                w1 = min(S, kb * 128 + 256)
                W = w1 - w0
                pss = mmps.tile([128, 384], F32, tag="mm")
                nc.tensor.matmul(
                    pss[:, 0:W], lhsT=kt_full[:, kb * 128:(kb + 1) * 128],
                    rhs=qt_full[:, w0:w1], start=True, stop=True)
                ex = epool.tile([128, 384], F32, tag="ex")
                nc.scalar.activation(ex[:, 0:W], pss[:, 0:W], AF.Exp,
                                     bias=pm_sb[:, kb:kb + 1], scale=1.0)
                qbs = [qb for qb in (kb - 1, kb, kb + 1) if 0 <= qb < N_KB]
                for qb in qbs:
                    c0 = qb * 128 - w0
                    if qb == kb - 1:  # delta=+1: keep qj >= ki
                        nc.gpsimd.affine_select(
                            out=ex[:, c0:c0 + 128], in_=ex[:, c0:c0 + 128],
                            compare_op=mybir.AluOpType.is_ge, fill=0.0,
                            base=0, channel_multiplier=-1, pattern=[[1, 128]])
                    elif qb == kb + 1:  # delta=-1: keep ki >= qj
                        nc.gpsimd.affine_select(
                            out=ex[:, c0:c0 + 128], in_=ex[:, c0:c0 + 128],
                            compare_op=mybir.AluOpType.is_ge, fill=0.0,
                            base=0, channel_multiplier=1, pattern=[[-1, 128]])
                for qb in qbs:
                    c0 = qb * 128 - w0
                    if qb not in ps_o:
                        ps_o[qb] = ops.tile([128, 65], F32, name="ps_o", tag="o")
                    nc.tensor.matmul(
                        ps_o[qb], lhsT=ex[:, c0:c0 + 128],
                        rhs=v_aug[:, kb, h * 65:(h + 1) * 65],
                        start=(kb == max(0, qb - 1)),
                        stop=(kb == min(N_KB - 1, qb + 1)))
                # finalize qb whose last contribution was this kb
                done = [kb - 1] if kb < N_KB - 1 else [kb - 1, kb]
                for qb in done:
                    if qb < 0:
                        continue
                    po = ps_o.pop(qb)
                    rec = rpool.tile([128, 1], F32, tag="rec")
                    nc.vector.reciprocal(rec, po[:, 64:65])
                    vt = vtpool.tile([128, 64], F32, tag="vt")
                    nc.vector.tensor_scalar_mul(vt, po[:, 0:64], rec[:, 0:1])
                    pst = tps.tile([64, 128], F32, tag="t")
                    nc.tensor.transpose(pst, vt, ident)
                    nc.scalar.copy(valsT[h][:, qb * 128:(qb + 1) * 128], pst)

        # ---------------- phase 3: output projection (partial) ----------------
        for tb in range(N_TBLK):
            ts = slice(tb * 512, (tb + 1) * 512)
            for eb in range(N_EBLK):
                pso = mmps.tile([128, 512], F32, tag="mm")
                for h in range(HPC):
                    nc.tensor.matmul(
                        pso, lhsT=wo_sb[:, h, eb * 128:(eb + 1) * 128],
                        rhs=valsT[h][:, ts], start=(h == 0), stop=(h == HPC - 1))
                osb = ospool.tile([128, 512], F32, tag="os")
                nc.vector.tensor_copy(osb, pso)
                nc.sync.dma_start(oT[eb * 128:(eb + 1) * 128, ts], osb)

    nc.compile()
    return nc


def _get_compiled():
    global _compiled
    if _compiled is None:
        _compiled = _build()
    return _compiled


def _pack_inputs(x, padding_mask, Wqkv, Wo):
    """Per-core input maps. Core c: batch c//4, heads 3*(c%4)+[0,1,2]."""
    in_maps = []
    scale = 1.0 / np.sqrt(np.float32(HD))
    for c in range(N_CORES):
        b, hg = divmod(c, 4)
        heads = [3 * hg, 3 * hg + 1, 3 * hg + 2]
        q_rows = [Wqkv[h * 3 * HD: h * 3 * HD + HD] * scale for h in heads]
        k_rows = [Wqkv[h * 3 * HD + HD: h * 3 * HD + 2 * HD] for h in heads]
        v_rows = [Wqkv[h * 3 * HD + 2 * HD: h * 3 * HD + 3 * HD] for h in heads]
        # M-tiles: [q0|q1] [k0|k1] [q2|k2]
        wqk_np = np.concatenate(
            [q_rows[0], q_rows[1], k_rows[0], k_rows[1], q_rows[2], k_rows[2]],
            axis=0)  # [384, 768]
        wqk_packed = np.ascontiguousarray(
            wqk_np.T.reshape(N_DBLK, 128, 384).transpose(1, 0, 2))
        wv_np = np.concatenate(v_rows, axis=0)  # [192, 768]
        wv_packed = np.ascontiguousarray(
            wv_np.T.reshape(N_DBLK, 128, 192).transpose(1, 0, 2))
        wo_packed = np.stack(
            [np.ascontiguousarray(Wo[:, h * HD:(h + 1) * HD].T) for h in heads],
            axis=1)  # [64, 3, 768]
        pm_add = np.where(padding_mask[b], 0.0, NEG).astype(np.float32)
        pm_packed = np.ascontiguousarray(pm_add.reshape(N_KB, 128).T)
        xT_b = np.ascontiguousarray(x[b].T)
        in_maps.append({
            "xT": xT_b.astype(np.float32),
            "wqk": wqk_packed.astype(np.float32),
            "wv": wv_packed.astype(np.float32),
            "wo": wo_packed.astype(np.float32),
            "pmask": pm_packed.astype(np.float32),
        })
    return in_maps


def _kernel_numpy(x, padding_mask, Wqkv, bqkv, Wo, bo):
    """Exact-math fallback (only used for unexpected inputs, e.g. bqkv != 0)."""
    B_, S_, D_ = x.shape
    hd = Wqkv.shape[0] // (3 * H)
    qkv = x @ Wqkv.T + bqkv
    qkv = qkv.reshape(B_, S_, H, 3 * hd).transpose(0, 2, 1, 3)
    q, k, v = np.split(qkv, 3, axis=-1)
    r = np.arange(S_)
    band = np.abs(r[:, None] - r[None, :]) <= HALF_WIN
    scores = np.einsum("bhqd,bhkd->bhqk", q, k) / np.sqrt(np.float32(hd))
    scores = np.where(band[None, None], scores, -np.inf)
    scores = np.where(padding_mask[:, None, None, :], scores, -np.inf)
    m = scores.max(axis=-1, keepdims=True)
    e = np.exp(scores - np.where(np.isfinite(m), m, 0.0))
    ssum = e.sum(axis=-1, keepdims=True)
    attn = np.where(ssum > 0, e / np.where(ssum > 0, ssum, 1.0), 0.0)
    vals = np.einsum("bhqk,bhkd->bhqd", attn, v)
    vals = vals.transpose(0, 2, 1, 3).reshape(B_, S_, H * hd)
    return (vals @ Wo.T + bo).astype(np.float32)


def kernel(**inputs):
    x = np.asarray(inputs["x"], dtype=np.float32)
    padding_mask = np.asarray(inputs["padding_mask"]).astype(bool)
    Wqkv = np.asarray(inputs["Wqkv"], dtype=np.float32)
    bqkv = np.asarray(inputs["bqkv"], dtype=np.float32)
    Wo = np.asarray(inputs["Wo"], dtype=np.float32)
    bo = np.asarray(inputs["bo"], dtype=np.float32)

    if x.shape != (B, S, D) or np.any(bqkv != 0.0):
        return _kernel_numpy(x, padding_mask, Wqkv, bqkv, Wo, bo)

    from concourse.bass_utils import run_bass_kernel_spmd

    nc = _get_compiled()
    in_maps = _pack_inputs(x, padding_mask, Wqkv, Wo)
    res = run_bass_kernel_spmd(nc, in_maps, core_ids=list(range(N_CORES)))
    out = np.zeros((B, S, E), dtype=np.float32)
    for c in range(N_CORES):
        b = c // 4
        out[b] += res.results[c]["oT"].T
    out += bo

    if not padding_mask.all():
        # degenerate rows: query t whose whole key window is masked -> o = bo
        for b in range(B):
            valid = padding_mask[b]
            for t in range(S):
                lo, hi = max(0, t - HALF_WIN), min(S, t + HALF_WIN + 1)
                if not valid[lo:hi].any():
                    out[b, t] = bo
    return out
